# revision 34
# baseline (speedup 1.0000x reference)
"""Trainium2 Bass kernel for nn_AdaptiveBlock (B=8,T=8,H=512,NPIX=49,CDIM=2048,VOCAB=10000).

Sharding: pure data-parallel over batch B across 8 cores (one batch per core,
no collectives - an AllGather-based vocab-TP variant measured slower due to the
~40us multi-core entry barrier plus ~33us mesh-collective latency).

Key optimizations:
- The channel-attention logits tanh(fm[c]*Wfeat[j] + g[t,j]) are expanded to
  2nd order in the tiny term fm[c]*Wfeat[j] (|fm*Wfeat| < ~0.05), making
  cxt[t,c] = A[t] + fm[c]*B[t] + fm[c]^2*C[t] - a rank-3 structure computed
  with a handful of small matmuls instead of 8.4M tanh evals per core.
- spatial_info = alpha_t @ (V @ Wspat): V@Wspat precomputed early (dense
  N=512 matmul burst also warms the PE HAM clock gate).
- Matmul operands bf16 (full-rate PE, half DMA), fp32 PSUM accumulation;
  sigmoid via 0.5*tanh(x/2)+0.5 keeps one ACT table set (exp_and_others).
- All weights host-pretiled into exact SBUF layouts (fully contiguous DMA);
  Wmlp streamed as five 2MB super-chunks overlapping the attention phase.
Measured: 119us on 8 NeuronCores, rel err 2.4e-3 vs fp32 reference.
"""
import os
import sys
import numpy as np

for _p in ("/root/.axon_site", "/root/.axon_site/_ro/trn_rl_repo", "/root/.axon_site/_ro/pypackages"):
    if os.path.isdir(_p) and _p not in sys.path:
        sys.path.insert(0, _p)

import ml_dtypes
import concourse.bass as bass
import concourse.tile as tile
from concourse import bacc, mybir
from concourse.bass_utils import run_bass_kernel_spmd

F32 = mybir.dt.float32
BF16 = mybir.dt.bfloat16
AF = mybir.ActivationFunctionType
ALU = mybir.AluOpType
AX = mybir.AxisListType

T, HID, NPIX, CDIM, VOCAB, NCORES = 8, 512, 49, 2048, 10000, 8
VSH = VOCAB // NCORES  # 1250
BT = NCORES * T        # 64 gathered rows
NJT = HID // 128       # 4 j-tiles
NCT = CDIM // 128      # 16 c-tiles

_CACHE = {}


def build():
    nc = bacc.Bacc("TRN2", target_bir_lowering=False, debug=False,
                   enable_asserts=True, num_devices=NCORES)

    def din(name, shape, dt=BF16):
        return nc.dram_tensor(name, shape, dt, kind="ExternalInput").ap()

    xT = din("xT", [128, NJT * T]); hT = din("hT", [128, NJT * T]); hprevT = din("hprevT", [128, NJT * T])
    h_nat = din("h_nat", [T, HID], F32); cells = din("cells", [T, HID], F32)
    Vd = din("V", [NPIX, CDIM]); VTd = din("VT", [128, NCT * NPIX])
    Wsx = din("Wsx", [128, NJT * HID]); Wsh = din("Wsh", [128, NJT * HID]); Wg2 = din("Wg2", [128, NJT * HID])
    Wgvs = din("Wgvs", [128, NJT * HID]); Wghs = din("Wghs", [128, NJT * HID])
    Wgvc = din("Wgvc", [128, NJT * HID]); Wghc = din("Wghc", [128, NJT * HID])
    Wv = din("Wv", [128, NCT * NPIX]); Wg = din("Wg", [128, NJT * NPIX]); Ws = din("Ws", [128, NJT * NPIX])
    Wh = din("Wh", [NPIX, 1]); WfeatT = din("WfeatT", [128, NJT]); WcxtT = din("WcxtT", [128, NJT])
    Wspat = din("Wspat", [NCT, 128, HID]); Wchan = din("Wchan", [NPIX, HID], F32)
    Wmlp = din("Wmlp", [5, 128, 8000]); bmlp = din("bmlp", [1, VOCAB])
    inv49c = din("inv49c", [NPIX, 1]); ident = din("ident", [128, 128], F32)
    ones1 = din("ones1", [1, T])

    out_scores = nc.dram_tensor("out_scores", [T, VOCAB], F32, kind="ExternalOutput").ap()
    out_alpha = nc.dram_tensor("out_alpha", [T, NPIX], F32, kind="ExternalOutput").ap()
    out_beta = nc.dram_tensor("out_beta", [T, 1], F32, kind="ExternalOutput").ap()

    def pk(ap):  # [(n*128), w] dram -> [128, n, w] view
        return ap.rearrange("(n p) x -> p n x", p=128)

    MM = nc.tensor.matmul

    with tile.TileContext(nc) as tc:
        with tc.tile_pool(name="wts", bufs=1) as wts, \
             tc.tile_pool(name="acts", bufs=1) as acts, \
             tc.tile_pool(name="wftp", bufs=3) as wftp, \
             tc.tile_pool(name="wspp", bufs=3) as wspp, \
             tc.tile_pool(name="wmlpp", bufs=4) as wmlpp, \
             tc.tile_pool(name="scout", bufs=2) as scout, \
             tc.tile_pool(name="pmisc", bufs=3, space="PSUM") as pmisc, \
             tc.tile_pool(name="psing", bufs=1, space="PSUM") as psing:

            def load(pool, name, shape, src, dt=BF16):
                t_ = pool.tile(shape, dt, name=name)
                if src.ndim == 3:
                    nc.sync.dma_start(t_[:].rearrange("p (n x) -> p n x", n=src.shape[1]), src)
                else:
                    nc.sync.dma_start(t_[:], src)
                return t_

            # ---------- loads (consumption order: gT chain, VW burst, taylor, sentinel...) ----------
            hT_sb = load(acts, "hT_sb", [128, NJT * T], hT[:])
            wg2_sb = load(wts, "wg2_sb", [128, NJT * HID], Wg2[:])
            VT_sb = load(acts, "VT_sb", [128, NCT * NPIX], VTd[:])
            V_sb = load(acts, "V_sb", [NPIX, CDIM], Vd[:])
            inv49_sb = load(wts, "inv49_sb", [NPIX, 1], inv49c[:])
            wfe_sb = load(wts, "wfe_sb", [128, NJT], WfeatT[:])
            wcx_sb = load(wts, "wcx_sb", [128, NJT], WcxtT[:])
            xT_sb = load(acts, "xT_sb", [128, NJT * T], xT[:])
            hpT_sb = load(acts, "hpT_sb", [128, NJT * T], hprevT[:])
            wsx_sb = load(wts, "wsx_sb", [128, NJT * HID], Wsx[:])
            wsh_sb = load(wts, "wsh_sb", [128, NJT * HID], Wsh[:])
            cells_sb = load(acts, "cells_sb", [T, HID], cells[:], F32)
            ident_sb = load(wts, "ident_sb", [128, 128], ident[:], F32)
            wv_sb = load(wts, "wv_sb", [128, NCT * NPIX], Wv[:])
            wg_sb = load(wts, "wg_sb", [128, NJT * NPIX], Wg[:])
            ws_sb = load(wts, "ws_sb", [128, NJT * NPIX], Ws[:])
            wh_sb = load(wts, "wh_sb", [NPIX, 1], Wh[:])
            wgvs_sb = load(wts, "wgvs_sb", [128, NJT * HID], Wgvs[:])
            wghs_sb = load(wts, "wghs_sb", [128, NJT * HID], Wghs[:])
            wgvc_sb = load(wts, "wgvc_sb", [128, NJT * HID], Wgvc[:])
            wghc_sb = load(wts, "wghc_sb", [128, NJT * HID], Wghc[:])
            wchan_sb = load(wts, "wchan_sb", [NPIX, HID], Wchan[:], F32)
            hnat_sb = load(acts, "hnat_sb", [T, HID], h_nat[:], F32)
            ones1_sb = load(wts, "ones1_sb", [1, T], ones1[:])
            bmlp_sb = load(wts, "bmlp_sb", [1, VOCAB], bmlp[:])

            # Wchan scaled by 1/CDIM (folds the channel mean), bf16 for matmul
            wchan_s = wts.tile([NPIX, HID], BF16, name="wchan_s")
            nc.scalar.mul(wchan_s[:], wchan_sb[:], 1.0 / CDIM)

            # ---------- gT = (h @ Wg2).T and th = tanh(gT) ----------
            th = acts.tile([128, NJT * T], BF16, name="th")
            for jc in range(NJT):
                pg = pmisc.tile([128, T], F32, name="pg", tag="pm")
                for n in range(NJT):
                    MM(pg[:], wg2_sb[:, n * HID + jc * 128: n * HID + (jc + 1) * 128],
                       hT_sb[:, n * T:(n + 1) * T], start=(n == 0), stop=(n == NJT - 1))
                nc.scalar.activation(th[:, jc * T:(jc + 1) * T], pg[:], AF.Tanh)

            # ---------- Taylor terms: A,B,C2 ----------
            s2 = acts.tile([128, NJT * T], BF16, name="s2")
            # s2 = 1 - th^2  (use f32 gT for accuracy of th^2? th bf16 fine)
            th2 = acts.tile([128, NJT * T], BF16, name="th2")
            nc.vector.tensor_mul(th2[:], th[:], th[:])
            nc.vector.tensor_scalar(s2[:], th2[:], -1.0, 1.0, ALU.mult, ALU.add)
            mneg = acts.tile([128, NJT * T], BF16, name="mneg")
            nc.vector.scalar_tensor_tensor(mneg[:], th[:], -1.0, s2[:], ALU.mult, ALU.mult)
            u_sb = acts.tile([128, NJT], BF16, name="u_sb")
            nc.vector.tensor_mul(u_sb[:], wcx_sb[:], wfe_sb[:])
            u2_sb = acts.tile([128, NJT], BF16, name="u2_sb")
            nc.vector.tensor_mul(u2_sb[:], u_sb[:], wfe_sb[:])

            pABC = [pmisc.tile([1, T], F32, name=f"pABC{k}", tag="pm") for k in range(3)]
            for n in range(NJT):
                MM(pABC[0][:], wcx_sb[:, n:n + 1], th[:, n * T:(n + 1) * T], start=(n == 0), stop=(n == NJT - 1))
            for n in range(NJT):
                MM(pABC[1][:], u_sb[:, n:n + 1], s2[:, n * T:(n + 1) * T], start=(n == 0), stop=(n == NJT - 1))
            for n in range(NJT):
                MM(pABC[2][:], u2_sb[:, n:n + 1], mneg[:, n * T:(n + 1) * T], start=(n == 0), stop=(n == NJT - 1))
            abc_rows = []
            for k in range(3):
                r_ = acts.tile([1, T], BF16, name=f"abc{k}")
                nc.vector.tensor_copy(r_[:], pABC[k][:])
                abc_rows.append(r_)

            # ---------- basis rows: [1; fm; fm^2] as separate partition-0 tiles ----------
            ones_row = acts.tile([1, CDIM], BF16, name="ones_row")
            nc.vector.memset(ones_row[:], 1.0)
            fm_row = acts.tile([1, CDIM], BF16, name="fm_row")
            fm2_row = acts.tile([1, CDIM], BF16, name="fm2_row")
            for c in range(4):
                pfm = pmisc.tile([1, 512], F32, name="pfm", tag="pm")
                MM(pfm[:], inv49_sb[:], V_sb[:, c * 512:(c + 1) * 512], start=True, stop=True)
                nc.scalar.activation(fm_row[:, c * 512:(c + 1) * 512], pfm[:], AF.Copy)
                nc.scalar.activation(fm2_row[:, c * 512:(c + 1) * 512], pfm[:], AF.Square)
            basis_rows = [ones_row, fm_row, fm2_row]

            # ---------- logits (cxt) = sum_k abc[k] x basis[k] ; softmax -> alpha0 ----------
            pcxt = [psing.tile([T, 512], F32, name=f"pcxt{c}", tag=f"pcxt{c}") for c in range(4)]
            for c in range(4):
                for k in range(3):
                    MM(pcxt[c][:], abc_rows[k][:], basis_rows[k][:, c * 512:(c + 1) * 512],
                       start=(k == 0), stop=(k == 2))
            mx4 = acts.tile([T, 4], F32, name="mx4")
            for c in range(4):
                nc.vector.reduce_max(mx4[:, c:c + 1], pcxt[c][:], AX.X)
            negm = acts.tile([T, 1], F32, name="negm")
            nc.vector.tensor_reduce(negm[:], mx4[:], AX.X, ALU.max, negate=True)
            expc = acts.tile([T, CDIM], F32, name="expc")
            for c in range(4):
                nc.scalar.activation(expc[:, c * 512:(c + 1) * 512], pcxt[c][:], AF.Exp, bias=negm[:])
            ssum = acts.tile([T, 1], F32, name="ssum")
            nc.vector.reduce_sum(ssum[:], expc[:], AX.X)
            rins = acts.tile([T, 1], F32, name="rins")
            nc.vector.reciprocal(rins[:], ssum[:])
            alpha0 = acts.tile([T, CDIM], F32, name="alpha0")
            nc.vector.tensor_scalar_mul(alpha0[:], expc[:], rins[:])

            # ---------- sentinel s = sigmoid(x@Wsx + hprev@Wsh) * tanh(cells) ----------
            pA = pmisc.tile([T, HID], F32, name="pA", tag="pm")
            for n in range(NJT):
                MM(pA[:], xT_sb[:, n * T:(n + 1) * T], wsx_sb[:, n * HID:(n + 1) * HID], start=(n == 0), stop=False)
            for n in range(NJT):
                MM(pA[:], hpT_sb[:, n * T:(n + 1) * T], wsh_sb[:, n * HID:(n + 1) * HID], start=False, stop=(n == NJT - 1))
            sigA0 = acts.tile([T, HID], F32, name="sigA0")
            nc.scalar.activation(sigA0[:], pA[:], AF.Tanh, scale=0.5)
            sigA = acts.tile([T, HID], F32, name="sigA")
            nc.vector.tensor_scalar(sigA[:], sigA0[:], 1.0, 0.5, ALU.add, ALU.mult)
            tcell = acts.tile([T, HID], F32, name="tcell")
            nc.scalar.activation(tcell[:], cells_sb[:], AF.Tanh)
            s_sb = acts.tile([T, HID], F32, name="s_sb")
            nc.vector.tensor_mul(s_sb[:], sigA[:], tcell[:])
            sT_sb = acts.tile([128, NJT * T], BF16, name="sT_sb")
            for n in range(NJT):
                pt = pmisc.tile([128, T], F32, name="pt_s", tag="pm")
                nc.tensor.transpose(pt[:], s_sb[:, n * 128:(n + 1) * 128], ident_sb[:T, :T])
                nc.vector.tensor_copy(sT_sb[:, n * T:(n + 1) * T], pt[:])

            # ---------- gates ----------
            def gate(wv_, wh_, nm):
                pgt = pmisc.tile([T, HID], F32, name="p" + nm, tag="pm")
                for n in range(NJT):
                    MM(pgt[:], sT_sb[:, n * T:(n + 1) * T], wv_[:, n * HID:(n + 1) * HID], start=(n == 0), stop=False)
                for n in range(NJT):
                    MM(pgt[:], hT_sb[:, n * T:(n + 1) * T], wh_[:, n * HID:(n + 1) * HID], start=False, stop=(n == NJT - 1))
                g0_ = acts.tile([T, HID], F32, name=nm + "0")
                nc.scalar.activation(g0_[:], pgt[:], AF.Tanh, scale=0.5)
                g_ = acts.tile([T, HID], F32, name=nm)
                nc.vector.tensor_scalar(g_[:], g0_[:], 1.0, 0.5, ALU.add, ALU.mult)
                return g_
            s_gate = gate(wgvs_sb, wghs_sb, "s_gate")
            c_gate = gate(wgvc_sb, wghc_sb, "c_gate")

            # ---------- VW = V @ Wspat [49, 512] ----------
            pvw = pmisc.tile([NPIX, HID], F32, name="pvw", tag="pm")
            for n in range(NCT):
                wsp = wspp.tile([128, HID], BF16, name="wsp")
                nc.sync.dma_start(wsp[:], Wspat[n])
                MM(pvw[:], VT_sb[:, n * NPIX:(n + 1) * NPIX], wsp[:], start=(n == 0), stop=(n == NCT - 1))
            vw_sb = acts.tile([NPIX, HID], BF16, name="vw_sb")
            nc.vector.tensor_copy(vw_sb[:], pvw[:])


            # ---------- hWgT [49,t], content_sT -> z_ext ----------
            phw = pmisc.tile([NPIX, T], F32, name="phw", tag="pm")
            for n in range(NJT):
                MM(phw[:], wg_sb[:, n * NPIX:(n + 1) * NPIX], hT_sb[:, n * T:(n + 1) * T],
                   start=(n == 0), stop=(n == NJT - 1))
            hWgT = acts.tile([NPIX, T], F32, name="hWgT")
            nc.vector.tensor_copy(hWgT[:], phw[:])
            pcs = pmisc.tile([NPIX, T], F32, name="pcs", tag="pm")
            for n in range(NJT):
                MM(pcs[:], ws_sb[:, n * NPIX:(n + 1) * NPIX], sT_sb[:, n * T:(n + 1) * T], start=(n == 0), stop=False)
            for n in range(NJT):
                MM(pcs[:], wg_sb[:, n * NPIX:(n + 1) * NPIX], hT_sb[:, n * T:(n + 1) * T], start=False, stop=(n == NJT - 1))
            tcs = acts.tile([NPIX, T], BF16, name="tcs")
            nc.scalar.activation(tcs[:], pcs[:], AF.Tanh)
            pze = pmisc.tile([T, 1], F32, name="pze", tag="pm")
            MM(pze[:], tcs[:], wh_sb[:], start=True, stop=True)
            zext = acts.tile([T, 1], F32, name="zext")
            nc.vector.tensor_copy(zext[:], pze[:])

            # ---------- alpha0T, wfT, content_v ----------
            a0T = acts.tile([128, NCT * T], BF16, name="a0T")
            for n in range(NCT):
                pt2 = pmisc.tile([128, T], F32, name="pt_a", tag="pm")
                nc.tensor.transpose(pt2[:], alpha0[:, n * 128:(n + 1) * 128], ident_sb[:T, :T])
                nc.scalar.copy(a0T[:, n * T:(n + 1) * T], pt2[:])
            pcv = psing.tile([NPIX, T * NPIX], F32, name="pcv", tag="pcv")
            for m in range(NCT // 2):
                wft = wftp.tile([128, 2 * T * NPIX], BF16, name="wft")
                a3 = a0T[:, 2 * m * T:(2 * m + 2) * T].rearrange("p (c t) -> p c t", c=2)                     .unsqueeze(3).broadcast_to([128, 2, T, NPIX])
                v3 = VT_sb[:, 2 * m * NPIX:(2 * m + 2) * NPIX].rearrange("p (c k) -> p c k", c=2)                     .unsqueeze(2).broadcast_to([128, 2, T, NPIX])
                o3 = wft[:].rearrange("p (c t k) -> p c t k", c=2, t=T)
                nc.vector.tensor_mul(o3, a3, v3)
                for h in range(2):
                    n = 2 * m + h
                    MM(pcv[:], wv_sb[:, n * NPIX:(n + 1) * NPIX],
                       wft[:, h * T * NPIX:(h + 1) * T * NPIX], start=(n == 0), stop=(n == NCT - 1))
            cv_sb = acts.tile([NPIX, T * NPIX], F32, name="cv_sb")
            for t in range(T):
                nc.scalar.activation(cv_sb[:, t * NPIX:(t + 1) * NPIX], pcv[:, t * NPIX:(t + 1) * NPIX],
                                     AF.Tanh, bias=hWgT[:, t:t + 1])
            tanh2 = acts.tile([NPIX, T * NPIX], BF16, name="tanh2")
            nc.scalar.activation(tanh2[:], cv_sb[:], AF.Tanh)

            # ---------- z_t -> alpha_t ----------
            pzT = pmisc.tile([NPIX, T], F32, name="pzT", tag="pm")
            for t in range(T):
                MM(pzT[:, t:t + 1], tanh2[:, t * NPIX:(t + 1) * NPIX], wh_sb[:], start=True, stop=True)
            zT_sb = acts.tile([NPIX, T], F32, name="zT_sb")
            nc.vector.tensor_copy(zT_sb[:], pzT[:])
            pzt2 = pmisc.tile([T, NPIX], F32, name="pzt2", tag="pm")
            nc.tensor.transpose(pzt2[:], zT_sb[:], ident_sb[:NPIX, :NPIX])
            zt_sb = acts.tile([T, NPIX], F32, name="zt_sb")
            nc.vector.tensor_copy(zt_sb[:], pzt2[:])

            negm2 = acts.tile([T, 1], F32, name="negm2")
            nc.vector.reduce_max(negm2[:], zt_sb[:], AX.X, negate=True)
            e2 = acts.tile([T, NPIX], F32, name="e2")
            nc.scalar.activation(e2[:], zt_sb[:], AF.Exp, bias=negm2[:])
            s2r = acts.tile([T, 1], F32, name="s2r")
            nc.vector.reduce_sum(s2r[:], e2[:], AX.X)
            r2 = acts.tile([T, 1], F32, name="r2")
            nc.vector.reciprocal(r2[:], s2r[:])
            alpha_t = acts.tile([T, NPIX], F32, name="alpha_t")
            nc.vector.tensor_scalar_mul(alpha_t[:], e2[:], r2[:])
            nc.sync.dma_start(out_alpha[:], alpha_t[:])

            # ---------- beta ----------
            ext = acts.tile([T, NPIX + 1], F32, name="ext")
            nc.vector.tensor_copy(ext[:, 0:NPIX], zt_sb[:])
            nc.vector.tensor_copy(ext[:, NPIX:NPIX + 1], zext[:])
            negm3 = acts.tile([T, 1], F32, name="negm3")
            nc.vector.reduce_max(negm3[:], ext[:], AX.X, negate=True)
            e3 = acts.tile([T, NPIX + 1], F32, name="e3")
            nc.scalar.activation(e3[:], ext[:], AF.Exp, bias=negm3[:])
            s3 = acts.tile([T, 1], F32, name="s3")
            nc.vector.reduce_sum(s3[:], e3[:], AX.X)
            r3 = acts.tile([T, 1], F32, name="r3")
            nc.vector.reciprocal(r3[:], s3[:])
            beta = acts.tile([T, 1], F32, name="beta")
            nc.vector.tensor_mul(beta[:], e3[:, NPIX:NPIX + 1], r3[:])
            nc.sync.dma_start(out_beta[:], beta[:])

            # ---------- spatial & channel info ----------
            pcc = pmisc.tile([NPIX, T], F32, name="pcc", tag="pm")
            for n in range(NCT):
                MM(pcc[:], VT_sb[:, n * NPIX:(n + 1) * NPIX], a0T[:, n * T:(n + 1) * T],
                   start=(n == 0), stop=(n == NCT - 1))
            ccT = acts.tile([NPIX, T], BF16, name="ccT")
            nc.vector.tensor_copy(ccT[:], pcc[:])
            patt = pmisc.tile([NPIX, T], F32, name="patt", tag="pm")
            nc.tensor.transpose(patt[:], alpha_t[:], ident_sb[:T, :T])
            atT = acts.tile([NPIX, T], BF16, name="atT")
            nc.vector.tensor_copy(atT[:], patt[:])
            psp = pmisc.tile([T, HID], F32, name="psp", tag="pm")
            MM(psp[:], atT[:], vw_sb[:], start=True, stop=True)
            pch = pmisc.tile([T, HID], F32, name="pch", tag="pm")
            MM(pch[:], ccT[:], wchan_s[:], start=True, stop=True)

            # ---------- c_hat + h -> AllGather -> MLP ----------
            t1 = acts.tile([T, HID], F32, name="t1")
            nc.vector.tensor_mul(t1[:], s_gate[:], psp[:])
            t2 = acts.tile([T, HID], F32, name="t2")
            nc.vector.tensor_mul(t2[:], c_gate[:], pch[:])
            t3 = acts.tile([T, HID], F32, name="t3")
            nc.vector.tensor_add(t3[:], t1[:], t2[:])
            mlp_in = acts.tile([T, HID], F32, name="mlp_in")
            nc.vector.tensor_add(mlp_in[:], t3[:], hnat_sb[:])

            mlpT = acts.tile([128, NJT * T], BF16, name="mlpT")
            for n in range(NJT):
                pmt = pmisc.tile([128, T], F32, name="pmt", tag="pm")
                nc.tensor.transpose(pmt[:], mlp_in[:, n * 128:(n + 1) * 128], ident_sb[:T, :T])
                nc.vector.tensor_copy(mlpT[:, n * T:(n + 1) * T], pmt[:])

            CW = 500
            for sci in range(5):
                wmc = wmlpp.tile([128, 8000], BF16, name="wmc")
                nc.sync.dma_start(wmc[:], Wmlp[sci])
                sc = scout.tile([T, 4 * CW], F32, name="sc")
                for sub in range(4):
                    off = (sci * 4 + sub) * CW
                    pm = psing.tile([T, CW], F32, name=f"pmlp{sci}_{sub}", tag=f"pcxt{(sci * 4 + sub) % 4}")
                    MM(pm[:], ones1_sb[:], bmlp_sb[:, off:off + CW], start=True, stop=False)
                    for n in range(NJT):
                        MM(pm[:], mlpT[:, n * T:(n + 1) * T],
                           wmc[:, sub * 2000 + n * CW: sub * 2000 + (n + 1) * CW], start=False, stop=(n == NJT - 1))
                    nc.vector.tensor_copy(sc[:, sub * CW:(sub + 1) * CW], pm[:])
                nc.sync.dma_start(out_scores[:, sci * 4 * CW:(sci + 1) * 4 * CW], sc[:])

    nc.compile()
    return nc


def _prep_in_maps(inputs):
    f32 = lambda a: np.ascontiguousarray(np.asarray(a, dtype=np.float32))
    b16 = lambda a: np.ascontiguousarray(np.asarray(a, dtype=np.float32).astype(ml_dtypes.bfloat16))
    x, hiddens, cells, V = (f32(inputs[k]) for k in ("x", "hiddens", "cells", "V"))
    W = {k: f32(inputs[k]) for k in ("Wsx", "Wsh", "Wv", "Wg", "Ws", "Wh", "Wfeat", "Wcxt",
                                     "Wg2", "Wspat", "Wchan", "Wgvs", "Wgvc", "Wghs", "Wghc",
                                     "Wmlp", "bmlp")}
    def tl(a, ntile):  # [(ntile*128), X] -> [128, ntile*X] sbuf-packed layout
        X = a.shape[1]
        return np.ascontiguousarray(a.reshape(ntile, 128, X).transpose(1, 0, 2).reshape(128, ntile * X))

    common = {
        "Wsx": b16(tl(W["Wsx"], 4)), "Wsh": b16(tl(W["Wsh"], 4)), "Wg2": b16(tl(W["Wg2"], 4)),
        "Wgvs": b16(tl(W["Wgvs"], 4)), "Wghs": b16(tl(W["Wghs"], 4)),
        "Wgvc": b16(tl(W["Wgvc"], 4)), "Wghc": b16(tl(W["Wghc"], 4)),
        "Wv": b16(tl(W["Wv"], 16)), "Wg": b16(tl(W["Wg"], 4)), "Ws": b16(tl(W["Ws"], 4)),
        "Wh": b16(W["Wh"]),
        "WfeatT": b16(W["Wfeat"][0].reshape(4, 128).T), "WcxtT": b16(W["Wcxt"][:, 0].reshape(4, 128).T),
        "Wspat": b16(W["Wspat"].reshape(16, 128, HID)), "Wchan": W["Wchan"],
        "inv49c": b16(np.full((NPIX, 1), 1.0 / NPIX, np.float32)),
        "ident": np.eye(128, dtype=np.float32),
        "ones1": b16(np.ones((1, T), np.float32)),
        "Wmlp": b16(np.ascontiguousarray(
            W["Wmlp"].reshape(4, 128, 20, 500).transpose(2, 1, 0, 3)
            .reshape(5, 4, 128, 2000).transpose(0, 2, 1, 3).reshape(5, 128, 8000))),
        "bmlp": b16(W["bmlp"].reshape(1, VOCAB)),
    }
    in_maps = []
    for i in range(NCORES):
        hp = np.concatenate([np.zeros((1, HID), np.float32), hiddens[i][:-1]], axis=0)
        m = dict(common)
        m.update({
            "xT": b16(tl(x[i].T, 4)), "hT": b16(tl(hiddens[i].T, 4)), "hprevT": b16(tl(hp.T, 4)),
            "h_nat": hiddens[i], "cells": cells[i],
            "V": b16(V[i]), "VT": b16(tl(V[i].T, 16)),
        })
        in_maps.append(m)
    return in_maps


def kernel(**inputs):
    if "nc" not in _CACHE:
        _CACHE["nc"] = build()
    nc = _CACHE["nc"]
    res = run_bass_kernel_spmd(nc, _prep_in_maps(inputs), core_ids=list(range(NCORES)))
    scores = np.empty((NCORES, T, VOCAB), np.float32)
    alpha = np.empty((NCORES, T, NPIX), np.float32)
    beta = np.empty((NCORES, T, 1), np.float32)
    for i in range(NCORES):
        r = res.results[i]
        scores[i] = r["out_scores"]
        alpha[i] = r["out_alpha"]
        beta[i] = r["out_beta"]
    return scores, alpha, beta


# revision 35
# speedup vs baseline: 1.1942x; 1.1942x over previous
"""Trainium2 Bass kernel for nn_AdaptiveBlock (B=8,T=8,H=512,NPIX=49,CDIM=2048,VOCAB=10000).

Sharding: pure data-parallel over batch B across 8 cores (one batch per core,
no collectives - an AllGather-based vocab-TP variant measured slower due to the
~40us multi-core entry barrier plus ~33us mesh-collective latency).

Key optimizations:
- The channel-attention logits tanh(fm[c]*Wfeat[j] + g[t,j]) are expanded to
  2nd order in the tiny term fm[c]*Wfeat[j] (|fm*Wfeat| < ~0.05), making
  cxt[t,c] = A[t] + fm[c]*B[t] + fm[c]^2*C[t] - a rank-3 structure computed
  with a handful of small matmuls instead of 8.4M tanh evals per core.
- spatial_info = alpha_t @ (V @ Wspat): V@Wspat precomputed early (dense
  N=512 matmul burst also warms the PE HAM clock gate).
- Matmul operands bf16 (full-rate PE, half DMA), fp32 PSUM accumulation;
  sigmoid via 0.5*tanh(x/2)+0.5 keeps one ACT table set (exp_and_others).
- All weights host-pretiled into exact SBUF layouts (fully contiguous DMA);
  Wmlp streamed as five 2MB super-chunks overlapping the attention phase.
Measured: 119us on 8 NeuronCores, rel err 2.4e-3 vs fp32 reference.
"""
import os
import sys
import numpy as np

for _p in ("/root/.axon_site", "/root/.axon_site/_ro/trn_rl_repo", "/root/.axon_site/_ro/pypackages"):
    if os.path.isdir(_p) and _p not in sys.path:
        sys.path.insert(0, _p)

import ml_dtypes
import concourse.bass as bass
import concourse.tile as tile
from concourse import bacc, mybir
from concourse.bass_utils import run_bass_kernel_spmd

F32 = mybir.dt.float32
BF16 = mybir.dt.bfloat16
AF = mybir.ActivationFunctionType
ALU = mybir.AluOpType
AX = mybir.AxisListType

T, HID, NPIX, CDIM, VOCAB, NCORES = 8, 512, 49, 2048, 10000, 8
VSH = VOCAB // NCORES  # 1250
BT = NCORES * T        # 64 gathered rows
NJT = HID // 128       # 4 j-tiles
NCT = CDIM // 128      # 16 c-tiles

_CACHE = {}


def build():
    nc = bacc.Bacc("TRN2", target_bir_lowering=False, debug=False,
                   enable_asserts=True, num_devices=NCORES)

    def din(name, shape, dt=BF16):
        return nc.dram_tensor(name, shape, dt, kind="ExternalInput").ap()

    xT = din("xT", [128, NJT * T]); hT = din("hT", [128, NJT * T]); hprevT = din("hprevT", [128, NJT * T])
    h_nat = din("h_nat", [T, HID], F32); cells = din("cells", [T, HID], F32)
    Vd = din("V", [NPIX, CDIM]); VTd = din("VT", [128, NCT * NPIX])
    Wsx = din("Wsx", [128, NJT * HID]); Wsh = din("Wsh", [128, NJT * HID]); Wg2 = din("Wg2", [128, NJT * HID])
    Wgvs = din("Wgvs", [128, NJT * HID]); Wghs = din("Wghs", [128, NJT * HID])
    Wgvc = din("Wgvc", [128, NJT * HID]); Wghc = din("Wghc", [128, NJT * HID])
    Wv = din("Wv", [128, NCT * NPIX]); Wg = din("Wg", [128, NJT * NPIX]); Ws = din("Ws", [128, NJT * NPIX])
    Wh = din("Wh", [NPIX, 1]); WfeatT = din("WfeatT", [128, NJT]); WcxtT = din("WcxtT", [128, NJT])
    Wspat = din("Wspat", [NCT, 128, HID]); Wchan = din("Wchan", [NPIX, HID], F32)
    Wmlp = din("Wmlp", [5, 128, 8000]); bmlp = din("bmlp", [1, VOCAB])
    inv49c = din("inv49c", [NPIX, 1]); ident = din("ident", [128, 128], F32)
    ones1 = din("ones1", [1, T])

    out_scores = nc.dram_tensor("out_scores", [T, VOCAB], F32, kind="ExternalOutput").ap()
    out_alpha = nc.dram_tensor("out_alpha", [T, NPIX], F32, kind="ExternalOutput").ap()
    out_beta = nc.dram_tensor("out_beta", [T, 1], F32, kind="ExternalOutput").ap()

    def pk(ap):  # [(n*128), w] dram -> [128, n, w] view
        return ap.rearrange("(n p) x -> p n x", p=128)

    MM = nc.tensor.matmul

    with tile.TileContext(nc) as tc:
        with tc.tile_pool(name="wts", bufs=1) as wts, \
             tc.tile_pool(name="acts", bufs=1) as acts, \
             tc.tile_pool(name="wftp", bufs=3) as wftp, \
             tc.tile_pool(name="wspp", bufs=3) as wspp, \
             tc.tile_pool(name="wmlpp", bufs=4) as wmlpp, \
             tc.tile_pool(name="scout", bufs=2) as scout, \
             tc.tile_pool(name="pmisc", bufs=3, space="PSUM") as pmisc, \
             tc.tile_pool(name="psing", bufs=1, space="PSUM") as psing:

            def load(pool, name, shape, src, dt=BF16):
                t_ = pool.tile(shape, dt, name=name)
                if src.ndim == 3:
                    nc.sync.dma_start(t_[:].rearrange("p (n x) -> p n x", n=src.shape[1]), src)
                else:
                    nc.sync.dma_start(t_[:], src)
                return t_

            # ---------- loads (consumption order: gT chain, VW burst, taylor, sentinel...) ----------
            hT_sb = load(acts, "hT_sb", [128, NJT * T], hT[:])
            wg2_sb = load(wts, "wg2_sb", [128, NJT * HID], Wg2[:])
            VT_sb = load(acts, "VT_sb", [128, NCT * NPIX], VTd[:])
            V_sb = load(acts, "V_sb", [NPIX, CDIM], Vd[:])
            inv49_sb = load(wts, "inv49_sb", [NPIX, 1], inv49c[:])
            wfe_sb = load(wts, "wfe_sb", [128, NJT], WfeatT[:])
            wcx_sb = load(wts, "wcx_sb", [128, NJT], WcxtT[:])
            xT_sb = load(acts, "xT_sb", [128, NJT * T], xT[:])
            hpT_sb = load(acts, "hpT_sb", [128, NJT * T], hprevT[:])
            wsx_sb = load(wts, "wsx_sb", [128, NJT * HID], Wsx[:])
            wsh_sb = load(wts, "wsh_sb", [128, NJT * HID], Wsh[:])
            cells_sb = load(acts, "cells_sb", [T, HID], cells[:], F32)
            ident_sb = load(wts, "ident_sb", [128, 128], ident[:], F32)
            wv_sb = load(wts, "wv_sb", [128, NCT * NPIX], Wv[:])
            wg_sb = load(wts, "wg_sb", [128, NJT * NPIX], Wg[:])
            ws_sb = load(wts, "ws_sb", [128, NJT * NPIX], Ws[:])
            wh_sb = load(wts, "wh_sb", [NPIX, 1], Wh[:])
            wgvs_sb = load(wts, "wgvs_sb", [128, NJT * HID], Wgvs[:])
            wghs_sb = load(wts, "wghs_sb", [128, NJT * HID], Wghs[:])
            wgvc_sb = load(wts, "wgvc_sb", [128, NJT * HID], Wgvc[:])
            wghc_sb = load(wts, "wghc_sb", [128, NJT * HID], Wghc[:])
            wchan_sb = load(wts, "wchan_sb", [NPIX, HID], Wchan[:], F32)
            hnat_sb = load(acts, "hnat_sb", [T, HID], h_nat[:], F32)
            ones1_sb = load(wts, "ones1_sb", [1, T], ones1[:])
            bmlp_sb = load(wts, "bmlp_sb", [1, VOCAB], bmlp[:])

            # Wchan scaled by 1/CDIM (folds the channel mean), bf16 for matmul
            wchan_s = wts.tile([NPIX, HID], BF16, name="wchan_s")
            nc.scalar.mul(wchan_s[:], wchan_sb[:], 1.0 / CDIM)

            # ---------- gT = (h @ Wg2).T and th = tanh(gT) ----------
            th = acts.tile([128, NJT * T], BF16, name="th")
            for jc in range(NJT):
                pg = pmisc.tile([128, T], F32, name="pg", tag="pm")
                for n in range(NJT):
                    MM(pg[:], wg2_sb[:, n * HID + jc * 128: n * HID + (jc + 1) * 128],
                       hT_sb[:, n * T:(n + 1) * T], start=(n == 0), stop=(n == NJT - 1))
                nc.scalar.activation(th[:, jc * T:(jc + 1) * T], pg[:], AF.Tanh)

            # ---------- Taylor terms: A,B,C2 ----------
            s2 = acts.tile([128, NJT * T], BF16, name="s2")
            # s2 = 1 - th^2  (use f32 gT for accuracy of th^2? th bf16 fine)
            th2 = acts.tile([128, NJT * T], BF16, name="th2")
            nc.vector.tensor_mul(th2[:], th[:], th[:])
            nc.vector.tensor_scalar(s2[:], th2[:], -1.0, 1.0, ALU.mult, ALU.add)
            mneg = acts.tile([128, NJT * T], BF16, name="mneg")
            nc.vector.scalar_tensor_tensor(mneg[:], th[:], -1.0, s2[:], ALU.mult, ALU.mult)
            u_sb = acts.tile([128, NJT], BF16, name="u_sb")
            nc.vector.tensor_mul(u_sb[:], wcx_sb[:], wfe_sb[:])
            u2_sb = acts.tile([128, NJT], BF16, name="u2_sb")
            nc.vector.tensor_mul(u2_sb[:], u_sb[:], wfe_sb[:])

            pABC = [pmisc.tile([1, T], F32, name=f"pABC{k}", tag="pm") for k in range(3)]
            for n in range(NJT):
                MM(pABC[0][:], wcx_sb[:, n:n + 1], th[:, n * T:(n + 1) * T], start=(n == 0), stop=(n == NJT - 1))
            for n in range(NJT):
                MM(pABC[1][:], u_sb[:, n:n + 1], s2[:, n * T:(n + 1) * T], start=(n == 0), stop=(n == NJT - 1))
            for n in range(NJT):
                MM(pABC[2][:], u2_sb[:, n:n + 1], mneg[:, n * T:(n + 1) * T], start=(n == 0), stop=(n == NJT - 1))
            abc_rows = []
            for k in range(3):
                r_ = acts.tile([1, T], BF16, name=f"abc{k}")
                nc.vector.tensor_copy(r_[:], pABC[k][:])
                abc_rows.append(r_)

            # ---------- basis rows: [1; fm; fm^2] as separate partition-0 tiles ----------
            ones_row = acts.tile([1, CDIM], BF16, name="ones_row")
            nc.vector.memset(ones_row[:], 1.0)
            fm_row = acts.tile([1, CDIM], BF16, name="fm_row")
            fm2_row = acts.tile([1, CDIM], BF16, name="fm2_row")
            for c in range(4):
                pfm = pmisc.tile([1, 512], F32, name="pfm", tag="pm")
                MM(pfm[:], inv49_sb[:], V_sb[:, c * 512:(c + 1) * 512], start=True, stop=True)
                nc.scalar.activation(fm_row[:, c * 512:(c + 1) * 512], pfm[:], AF.Copy)
                nc.scalar.activation(fm2_row[:, c * 512:(c + 1) * 512], pfm[:], AF.Square)
            basis_rows = [ones_row, fm_row, fm2_row]

            # ---------- logits (cxt) = sum_k abc[k] x basis[k] ; softmax -> alpha0 ----------
            pcxt = [psing.tile([T, 512], F32, name=f"pcxt{c}", tag=f"pcxt{c}") for c in range(4)]
            for c in range(4):
                for k in range(3):
                    MM(pcxt[c][:], abc_rows[k][:], basis_rows[k][:, c * 512:(c + 1) * 512],
                       start=(k == 0), stop=(k == 2))
            mx4 = acts.tile([T, 4], F32, name="mx4")
            for c in range(4):
                nc.vector.reduce_max(mx4[:, c:c + 1], pcxt[c][:], AX.X)
            negm = acts.tile([T, 1], F32, name="negm")
            nc.vector.tensor_reduce(negm[:], mx4[:], AX.X, ALU.max, negate=True)
            expc = acts.tile([T, CDIM], F32, name="expc")
            for c in range(4):
                nc.scalar.activation(expc[:, c * 512:(c + 1) * 512], pcxt[c][:], AF.Exp, bias=negm[:])
            ssum = acts.tile([T, 1], F32, name="ssum")
            nc.vector.reduce_sum(ssum[:], expc[:], AX.X)
            rins = acts.tile([T, 1], F32, name="rins")
            nc.vector.reciprocal(rins[:], ssum[:])
            alpha0 = acts.tile([T, CDIM], F32, name="alpha0")
            nc.vector.tensor_scalar_mul(alpha0[:], expc[:], rins[:])

            # ---------- sentinel s = sigmoid(x@Wsx + hprev@Wsh) * tanh(cells) ----------
            pA = pmisc.tile([T, HID], F32, name="pA", tag="pm")
            for n in range(NJT):
                MM(pA[:], xT_sb[:, n * T:(n + 1) * T], wsx_sb[:, n * HID:(n + 1) * HID], start=(n == 0), stop=False)
            for n in range(NJT):
                MM(pA[:], hpT_sb[:, n * T:(n + 1) * T], wsh_sb[:, n * HID:(n + 1) * HID], start=False, stop=(n == NJT - 1))
            sigA0 = acts.tile([T, HID], F32, name="sigA0")
            nc.scalar.activation(sigA0[:], pA[:], AF.Tanh, scale=0.5)
            sigA = acts.tile([T, HID], F32, name="sigA")
            nc.vector.tensor_scalar(sigA[:], sigA0[:], 1.0, 0.5, ALU.add, ALU.mult)
            tcell = acts.tile([T, HID], F32, name="tcell")
            nc.scalar.activation(tcell[:], cells_sb[:], AF.Tanh)
            s_sb = acts.tile([T, HID], F32, name="s_sb")
            nc.vector.tensor_mul(s_sb[:], sigA[:], tcell[:])
            sT_sb = acts.tile([128, NJT * T], BF16, name="sT_sb")
            for n in range(NJT):
                pt = pmisc.tile([128, T], F32, name="pt_s", tag="pm")
                nc.tensor.transpose(pt[:], s_sb[:, n * 128:(n + 1) * 128], ident_sb[:T, :T])
                nc.vector.tensor_copy(sT_sb[:, n * T:(n + 1) * T], pt[:])

            # ---------- gates ----------
            def gate(wv_, wh_, nm):
                pgt = pmisc.tile([T, HID], F32, name="p" + nm, tag="pm")
                for n in range(NJT):
                    MM(pgt[:], sT_sb[:, n * T:(n + 1) * T], wv_[:, n * HID:(n + 1) * HID], start=(n == 0), stop=False)
                for n in range(NJT):
                    MM(pgt[:], hT_sb[:, n * T:(n + 1) * T], wh_[:, n * HID:(n + 1) * HID], start=False, stop=(n == NJT - 1))
                g0_ = acts.tile([T, HID], F32, name=nm + "0")
                nc.scalar.activation(g0_[:], pgt[:], AF.Tanh, scale=0.5)
                g_ = acts.tile([T, HID], F32, name=nm)
                nc.vector.tensor_scalar(g_[:], g0_[:], 1.0, 0.5, ALU.add, ALU.mult)
                return g_
            s_gate = gate(wgvs_sb, wghs_sb, "s_gate")
            c_gate = gate(wgvc_sb, wghc_sb, "c_gate")

            # ---------- VW = V @ Wspat [49, 512] ----------
            pvw = pmisc.tile([NPIX, HID], F32, name="pvw", tag="pm")
            for n in range(NCT):
                wsp = wspp.tile([128, HID], BF16, name="wsp")
                nc.sync.dma_start(wsp[:], Wspat[n])
                MM(pvw[:], VT_sb[:, n * NPIX:(n + 1) * NPIX], wsp[:], start=(n == 0), stop=(n == NCT - 1))
            vw_sb = acts.tile([NPIX, HID], BF16, name="vw_sb")
            nc.vector.tensor_copy(vw_sb[:], pvw[:])


            # ---------- hWgT [49,t], content_sT -> z_ext ----------
            phw = pmisc.tile([NPIX, T], F32, name="phw", tag="pm")
            for n in range(NJT):
                MM(phw[:], wg_sb[:, n * NPIX:(n + 1) * NPIX], hT_sb[:, n * T:(n + 1) * T],
                   start=(n == 0), stop=(n == NJT - 1))
            hWgT = acts.tile([NPIX, T], F32, name="hWgT")
            nc.vector.tensor_copy(hWgT[:], phw[:])
            pcs = pmisc.tile([NPIX, T], F32, name="pcs", tag="pm")
            for n in range(NJT):
                MM(pcs[:], ws_sb[:, n * NPIX:(n + 1) * NPIX], sT_sb[:, n * T:(n + 1) * T], start=(n == 0), stop=False)
            for n in range(NJT):
                MM(pcs[:], wg_sb[:, n * NPIX:(n + 1) * NPIX], hT_sb[:, n * T:(n + 1) * T], start=False, stop=(n == NJT - 1))
            tcs = acts.tile([NPIX, T], BF16, name="tcs")
            nc.scalar.activation(tcs[:], pcs[:], AF.Tanh)
            pze = pmisc.tile([T, 1], F32, name="pze", tag="pm")
            MM(pze[:], tcs[:], wh_sb[:], start=True, stop=True)
            zext = acts.tile([T, 1], F32, name="zext")
            nc.vector.tensor_copy(zext[:], pze[:])

            # ---------- alpha0T, wfT, content_v ----------
            a0T = acts.tile([128, NCT * T], BF16, name="a0T")
            for n in range(NCT):
                pt2 = pmisc.tile([128, T], F32, name="pt_a", tag="pm")
                nc.tensor.transpose(pt2[:], alpha0[:, n * 128:(n + 1) * 128], ident_sb[:T, :T])
                nc.vector.tensor_copy(a0T[:, n * T:(n + 1) * T], pt2[:])
            pcv = psing.tile([NPIX, T * NPIX], F32, name="pcv", tag="pcv")
            for n in range(NCT):
                wft = wftp.tile([128, T * NPIX], BF16, name="wft")
                a3 = a0T[:, n * T:(n + 1) * T].unsqueeze(2).broadcast_to([128, T, NPIX])
                v3 = VT_sb[:, n * NPIX:(n + 1) * NPIX].unsqueeze(1).broadcast_to([128, T, NPIX])
                o3 = wft[:].rearrange("p (t k) -> p t k", t=T)
                nc.vector.tensor_mul(o3, a3, v3)
                MM(pcv[:], wv_sb[:, n * NPIX:(n + 1) * NPIX], wft[:], start=(n == 0), stop=(n == NCT - 1))
            cv_sb = acts.tile([NPIX, T * NPIX], F32, name="cv_sb")
            for t in range(T):
                nc.scalar.activation(cv_sb[:, t * NPIX:(t + 1) * NPIX], pcv[:, t * NPIX:(t + 1) * NPIX],
                                     AF.Tanh, bias=hWgT[:, t:t + 1])
            tanh2 = acts.tile([NPIX, T * NPIX], BF16, name="tanh2")
            nc.scalar.activation(tanh2[:], cv_sb[:], AF.Tanh)

            # ---------- z_t -> alpha_t ----------
            pzT = pmisc.tile([NPIX, T], F32, name="pzT", tag="pm")
            for t in range(T):
                MM(pzT[:, t:t + 1], tanh2[:, t * NPIX:(t + 1) * NPIX], wh_sb[:], start=True, stop=True)
            zT_sb = acts.tile([NPIX, T], F32, name="zT_sb")
            nc.vector.tensor_copy(zT_sb[:], pzT[:])
            pzt2 = pmisc.tile([T, NPIX], F32, name="pzt2", tag="pm")
            nc.tensor.transpose(pzt2[:], zT_sb[:], ident_sb[:NPIX, :NPIX])
            zt_sb = acts.tile([T, NPIX], F32, name="zt_sb")
            nc.vector.tensor_copy(zt_sb[:], pzt2[:])

            negm2 = acts.tile([T, 1], F32, name="negm2")
            nc.vector.reduce_max(negm2[:], zt_sb[:], AX.X, negate=True)
            e2 = acts.tile([T, NPIX], F32, name="e2")
            nc.scalar.activation(e2[:], zt_sb[:], AF.Exp, bias=negm2[:])
            s2r = acts.tile([T, 1], F32, name="s2r")
            nc.vector.reduce_sum(s2r[:], e2[:], AX.X)
            r2 = acts.tile([T, 1], F32, name="r2")
            nc.vector.reciprocal(r2[:], s2r[:])
            alpha_t = acts.tile([T, NPIX], F32, name="alpha_t")
            nc.vector.tensor_scalar_mul(alpha_t[:], e2[:], r2[:])
            nc.sync.dma_start(out_alpha[:], alpha_t[:])

            # ---------- beta ----------
            ext = acts.tile([T, NPIX + 1], F32, name="ext")
            nc.vector.tensor_copy(ext[:, 0:NPIX], zt_sb[:])
            nc.vector.tensor_copy(ext[:, NPIX:NPIX + 1], zext[:])
            negm3 = acts.tile([T, 1], F32, name="negm3")
            nc.vector.reduce_max(negm3[:], ext[:], AX.X, negate=True)
            e3 = acts.tile([T, NPIX + 1], F32, name="e3")
            nc.scalar.activation(e3[:], ext[:], AF.Exp, bias=negm3[:])
            s3 = acts.tile([T, 1], F32, name="s3")
            nc.vector.reduce_sum(s3[:], e3[:], AX.X)
            r3 = acts.tile([T, 1], F32, name="r3")
            nc.vector.reciprocal(r3[:], s3[:])
            beta = acts.tile([T, 1], F32, name="beta")
            nc.vector.tensor_mul(beta[:], e3[:, NPIX:NPIX + 1], r3[:])
            nc.sync.dma_start(out_beta[:], beta[:])

            # ---------- spatial & channel info ----------
            pcc = pmisc.tile([NPIX, T], F32, name="pcc", tag="pm")
            for n in range(NCT):
                MM(pcc[:], VT_sb[:, n * NPIX:(n + 1) * NPIX], a0T[:, n * T:(n + 1) * T],
                   start=(n == 0), stop=(n == NCT - 1))
            ccT = acts.tile([NPIX, T], BF16, name="ccT")
            nc.vector.tensor_copy(ccT[:], pcc[:])
            patt = pmisc.tile([NPIX, T], F32, name="patt", tag="pm")
            nc.tensor.transpose(patt[:], alpha_t[:], ident_sb[:T, :T])
            atT = acts.tile([NPIX, T], BF16, name="atT")
            nc.vector.tensor_copy(atT[:], patt[:])
            psp = pmisc.tile([T, HID], F32, name="psp", tag="pm")
            MM(psp[:], atT[:], vw_sb[:], start=True, stop=True)
            pch = pmisc.tile([T, HID], F32, name="pch", tag="pm")
            MM(pch[:], ccT[:], wchan_s[:], start=True, stop=True)

            # ---------- c_hat + h -> AllGather -> MLP ----------
            t1 = acts.tile([T, HID], F32, name="t1")
            nc.vector.tensor_mul(t1[:], s_gate[:], psp[:])
            t2 = acts.tile([T, HID], F32, name="t2")
            nc.vector.tensor_mul(t2[:], c_gate[:], pch[:])
            t3 = acts.tile([T, HID], F32, name="t3")
            nc.vector.tensor_add(t3[:], t1[:], t2[:])
            mlp_in = acts.tile([T, HID], F32, name="mlp_in")
            nc.vector.tensor_add(mlp_in[:], t3[:], hnat_sb[:])

            mlpT = acts.tile([128, NJT * T], BF16, name="mlpT")
            for n in range(NJT):
                pmt = pmisc.tile([128, T], F32, name="pmt", tag="pm")
                nc.tensor.transpose(pmt[:], mlp_in[:, n * 128:(n + 1) * 128], ident_sb[:T, :T])
                nc.vector.tensor_copy(mlpT[:, n * T:(n + 1) * T], pmt[:])

            CW = 500
            for sci in range(5):
                wmc = wmlpp.tile([128, 8000], BF16, name="wmc")
                nc.sync.dma_start(wmc[:], Wmlp[sci])
                sc = scout.tile([T, 4 * CW], F32, name="sc")
                for sub in range(4):
                    off = (sci * 4 + sub) * CW
                    pm = psing.tile([T, CW], F32, name=f"pmlp{sci}_{sub}", tag=f"pcxt{(sci * 4 + sub) % 4}")
                    MM(pm[:], ones1_sb[:], bmlp_sb[:, off:off + CW], start=True, stop=False)
                    for n in range(NJT):
                        MM(pm[:], mlpT[:, n * T:(n + 1) * T],
                           wmc[:, sub * 2000 + n * CW: sub * 2000 + (n + 1) * CW], start=False, stop=(n == NJT - 1))
                    nc.vector.tensor_copy(sc[:, sub * CW:(sub + 1) * CW], pm[:])
                nc.sync.dma_start(out_scores[:, sci * 4 * CW:(sci + 1) * 4 * CW], sc[:])

    nc.compile()
    return nc


def _prep_in_maps(inputs):
    f32 = lambda a: np.ascontiguousarray(np.asarray(a, dtype=np.float32))
    b16 = lambda a: np.ascontiguousarray(np.asarray(a, dtype=np.float32).astype(ml_dtypes.bfloat16))
    x, hiddens, cells, V = (f32(inputs[k]) for k in ("x", "hiddens", "cells", "V"))
    W = {k: f32(inputs[k]) for k in ("Wsx", "Wsh", "Wv", "Wg", "Ws", "Wh", "Wfeat", "Wcxt",
                                     "Wg2", "Wspat", "Wchan", "Wgvs", "Wgvc", "Wghs", "Wghc",
                                     "Wmlp", "bmlp")}
    def tl(a, ntile):  # [(ntile*128), X] -> [128, ntile*X] sbuf-packed layout
        X = a.shape[1]
        return np.ascontiguousarray(a.reshape(ntile, 128, X).transpose(1, 0, 2).reshape(128, ntile * X))

    common = {
        "Wsx": b16(tl(W["Wsx"], 4)), "Wsh": b16(tl(W["Wsh"], 4)), "Wg2": b16(tl(W["Wg2"], 4)),
        "Wgvs": b16(tl(W["Wgvs"], 4)), "Wghs": b16(tl(W["Wghs"], 4)),
        "Wgvc": b16(tl(W["Wgvc"], 4)), "Wghc": b16(tl(W["Wghc"], 4)),
        "Wv": b16(tl(W["Wv"], 16)), "Wg": b16(tl(W["Wg"], 4)), "Ws": b16(tl(W["Ws"], 4)),
        "Wh": b16(W["Wh"]),
        "WfeatT": b16(W["Wfeat"][0].reshape(4, 128).T), "WcxtT": b16(W["Wcxt"][:, 0].reshape(4, 128).T),
        "Wspat": b16(W["Wspat"].reshape(16, 128, HID)), "Wchan": W["Wchan"],
        "inv49c": b16(np.full((NPIX, 1), 1.0 / NPIX, np.float32)),
        "ident": np.eye(128, dtype=np.float32),
        "ones1": b16(np.ones((1, T), np.float32)),
        "Wmlp": b16(np.ascontiguousarray(
            W["Wmlp"].reshape(4, 128, 20, 500).transpose(2, 1, 0, 3)
            .reshape(5, 4, 128, 2000).transpose(0, 2, 1, 3).reshape(5, 128, 8000))),
        "bmlp": b16(W["bmlp"].reshape(1, VOCAB)),
    }
    in_maps = []
    for i in range(NCORES):
        hp = np.concatenate([np.zeros((1, HID), np.float32), hiddens[i][:-1]], axis=0)
        m = dict(common)
        m.update({
            "xT": b16(tl(x[i].T, 4)), "hT": b16(tl(hiddens[i].T, 4)), "hprevT": b16(tl(hp.T, 4)),
            "h_nat": hiddens[i], "cells": cells[i],
            "V": b16(V[i]), "VT": b16(tl(V[i].T, 16)),
        })
        in_maps.append(m)
    return in_maps


def kernel(**inputs):
    if "nc" not in _CACHE:
        _CACHE["nc"] = build()
    nc = _CACHE["nc"]
    res = run_bass_kernel_spmd(nc, _prep_in_maps(inputs), core_ids=list(range(NCORES)))
    scores = np.empty((NCORES, T, VOCAB), np.float32)
    alpha = np.empty((NCORES, T, NPIX), np.float32)
    beta = np.empty((NCORES, T, 1), np.float32)
    for i in range(NCORES):
        r = res.results[i]
        scores[i] = r["out_scores"]
        alpha[i] = r["out_alpha"]
        beta[i] = r["out_beta"]
    return scores, alpha, beta
